# revision 9
# baseline (speedup 1.0000x reference)
"""Trainium2 Bass kernel for nn_MultiHeadAttention (B=2, S=4096, D=512, H=8).

Sharding: 8 cores = (2 batches) x (4 head-pairs). Each core computes two
heads' attention for one batch plus its partial output projection.

On-chip orientation is "k-major": S^T[k,q] = K @ Q^T is computed with k on
partitions, so softmax row-sums reduce over the partition axis — fused into
the context matmul via a ones column in V — and the context matmul needs no
transposes at all. The padding mask is folded into the score matmul as an
extra contraction row (lhsT row 64 = penalty, rhs row 64 = 1.0). exp runs on
ScalarE directly out of PSUM. attn is written to HBM in [k,q] layout and
transposed on the host during unsharding.

Matmul dtypes: projections fp32 (exact), scores/context/output fp32r
(TF32-like, ~2e-4 rel err, 4x faster than fp32).
"""

import os
import sys

sys.path.insert(0, "/opt/trn_rl_repo")

import numpy as np

import concourse.bass as bass
import concourse.mybir as mybir
import concourse.tile as tile
from concourse import bacc
from concourse.bass_utils import run_bass_kernel_spmd
import concourse.bass_utils as bass_utils

# Avoid S3 artifact uploads from the profiling path.
bass_utils.upload_artifacts = lambda tmpdir: f"file://{tmpdir}"

B = 2
S = 4096
D = 512
H = 8
DK = 64
HPC = 2          # heads per core
NC_CHUNKS = 4    # D / 128 contraction chunks
NKT = S // 128   # 32 k-tiles
SQB = 256        # phase-B q strip width
NSTRIP = S // SQB
PBLK = 512       # phase-A projection q block
KTQ = 4          # k-tiles per PSUM tile (exp batch)

F32 = mybir.dt.float32
F32R = mybir.dt.float32r

_NC_CACHE = None


def _f32(ap):
    return ap.bitcast(F32)


def _bcast_mid(ap, n):
    """Insert a step-0 middle free dim of extent n into a 2D AP."""
    return bass.AP(tensor=ap.tensor, offset=ap.offset, ap=[ap.ap[0], [0, n], ap.ap[1]])


def build_nc():
    nc = bacc.Bacc("TRN2", target_bir_lowering=False, debug=False, num_devices=8)

    qT = nc.dram_tensor("qT", [128, NC_CHUNKS, S], F32, kind="ExternalInput")
    kT = nc.dram_tensor("kT", [128, NC_CHUNKS, S], F32, kind="ExternalInput")
    vT = nc.dram_tensor("vT", [128, NC_CHUNKS, S], F32, kind="ExternalInput")
    wq = nc.dram_tensor("wq", [128, NC_CHUNKS, HPC, DK], F32, kind="ExternalInput")
    wk = nc.dram_tensor("wk", [128, NC_CHUNKS, HPC, DK], F32, kind="ExternalInput")
    wv = nc.dram_tensor("wv", [128, NC_CHUNKS, HPC, DK], F32, kind="ExternalInput")
    wo = nc.dram_tensor("wo", [DK, HPC, NC_CHUNKS, 128], F32R, kind="ExternalInput")
    # aux[0] = ones (Q^T row 64), aux[1] = mask penalty (K^T row 64)
    aux = nc.dram_tensor("aux", [2, 1, HPC, S], F32R, kind="ExternalInput")

    attn_t = nc.dram_tensor("attn_t", [HPC, NKT, 128, S], F32, kind="ExternalOutput")
    out_t = nc.dram_tensor("out_t", [NC_CHUNKS, 128, S], F32, kind="ExternalOutput")

    from contextlib import ExitStack
    with tile.TileContext(nc) as tc, ExitStack() as stack:
        # ---- persistent SBUF ----
        persist = stack.enter_context(tc.tile_pool(name="persist", bufs=1))
        QTp = persist.tile([65, HPC, S], F32R)   # rows 0-63 Q^T, row 64 ones
        KTp = persist.tile([65, HPC, S], F32R)   # rows 0-63 K^T, row 64 penalty
        Vsb = persist.tile([128, NKT, HPC, 65], F32R)  # cols 0-63 V, col 64 ones
        ctx_sb = persist.tile([DK, HPC, S], F32R)
        wq_sb = persist.tile([128, NC_CHUNKS, HPC, DK], F32)
        wk_sb = persist.tile([128, NC_CHUNKS, HPC, DK], F32)
        wv_sb = persist.tile([128, NC_CHUNKS, HPC, DK], F32)
        wo_sb = persist.tile([DK, HPC, NC_CHUNKS, 128], F32R)
        ones_row = persist.tile([1, 128], F32R)

        nc.scalar.dma_start(out=wq_sb[:], in_=wq[:])
        nc.scalar.dma_start(out=wk_sb[:], in_=wk[:])
        nc.scalar.dma_start(out=wv_sb[:], in_=wv[:])
        nc.scalar.dma_start(out=wo_sb[:], in_=wo[:])
        nc.scalar.dma_start(out=QTp[64:65, :, :], in_=aux[0, :, :, :])
        nc.scalar.dma_start(out=KTp[64:65, :, :], in_=aux[1, :, :, :])
        nc.vector.memset(_f32(ones_row[:]), 1.0)
        nc.vector.memset(_f32(Vsb[:, :, :, 64:65]), 1.0)

        # ---- phase A: projections ----
        with tc.tile_pool(name="stage", bufs=4) as stage, \
             tc.tile_pool(name="pps", bufs=3, space="PSUM") as pps:
            for (src, w_sb, dstp) in ((qT, wq_sb, QTp), (kT, wk_sb, KTp)):
                chunks = []
                for c in range(NC_CHUNKS):
                    t = stage.tile([128, S], F32, tag="stage")
                    nc.scalar.dma_start(out=t[:], in_=src[:, c, :])
                    chunks.append(t)
                for h in range(HPC):
                    for pb in range(S // PBLK):
                        ps = pps.tile([DK, PBLK], F32, tag="proj")
                        for c in range(NC_CHUNKS):
                            nc.tensor.matmul(
                                ps[:],
                                w_sb[:, c, h, :],
                                chunks[c][:, pb * PBLK:(pb + 1) * PBLK],
                                start=(c == 0), stop=(c == NC_CHUNKS - 1),
                            )
                        nc.vector.tensor_copy(
                            out=dstp[0:DK, h, pb * PBLK:(pb + 1) * PBLK], in_=ps[:]
                        )
            # V = value @ Wv^T, built [k, dv] with k on partitions
            chunks = []
            for c in range(NC_CHUNKS):
                t = stage.tile([128, S], F32, tag="stage")
                nc.scalar.dma_start(out=t[:], in_=vT[:, c, :])
                chunks.append(t)
            for kt in range(NKT):
                ps = pps.tile([128, HPC * DK], F32, tag="vproj")
                for c in range(NC_CHUNKS):
                    nc.tensor.matmul(
                        ps[:],
                        chunks[c][:, kt * 128:(kt + 1) * 128],
                        wv_sb[:, c, :, :],
                        start=(c == 0), stop=(c == NC_CHUNKS - 1),
                    )
                for h in range(HPC):
                    nc.vector.tensor_copy(
                        out=Vsb[:, kt, h, 0:DK], in_=ps[:, h * DK:(h + 1) * DK]
                    )

        # ---- phase B: scores, softmax, context ----
        with tc.tile_pool(name="epool", bufs=2) as epool, \
             tc.tile_pool(name="small", bufs=3) as small, \
             tc.tile_pool(name="invp", bufs=2) as invp, \
             tc.tile_pool(name="sps", bufs=2, space="PSUM") as sps, \
             tc.tile_pool(name="cps", bufs=2, space="PSUM") as cps, \
             tc.tile_pool(name="ips", bufs=2, space="PSUM") as ips:
            for h in range(HPC):
                for sb in range(NSTRIP):
                    q0 = sb * SQB
                    E = epool.tile([128, NKT, SQB], F32R, tag="E")
                    cx = cps.tile([65, SQB], F32, tag="ctx")
                    for ktq in range(NKT // KTQ):
                        sp = sps.tile([128, KTQ, SQB], F32, tag="s")
                        for j in range(KTQ):
                            kt = ktq * KTQ + j
                            nc.tensor.matmul(
                                sp[:, j, :],
                                KTp[:, h, kt * 128:(kt + 1) * 128],
                                QTp[:, h, q0:q0 + SQB],
                                start=True, stop=True,
                            )
                        nc.scalar.activation(
                            out=E[:, ktq * KTQ:(ktq + 1) * KTQ, :],
                            in_=sp[:],
                            func=mybir.ActivationFunctionType.Exp,
                        )
                        for j in range(KTQ):
                            kt = ktq * KTQ + j
                            nc.tensor.matmul(
                                cx[:],
                                Vsb[:, kt, h, 0:65],
                                E[:, kt, :],
                                start=(kt == 0), stop=(kt == NKT - 1),
                            )
                    inv = small.tile([1, SQB], F32R, tag="inv")
                    with nc.allow_low_precision(reason="f32r inv for f32r matmul"):
                        nc.vector.reciprocal(out=inv[:], in_=cx[64:65, :])
                    ib = ips.tile([128, SQB], F32, tag="ib")
                    nc.tensor.matmul(ib[:], ones_row[:], inv[:], start=True, stop=True)
                    ibs = invp.tile([128, SQB], F32, tag="ibs")
                    nc.vector.tensor_copy(out=ibs[:], in_=ib[:])
                    with nc.allow_low_precision(reason="f32r P/ctx tiles"):
                        nc.vector.tensor_mul(
                            ctx_sb[:, h, q0:q0 + SQB], cx[0:DK, :], ibs[0:DK, :]
                        )
                        nc.vector.tensor_mul(
                            E[:], _f32(E[:]), _bcast_mid(ibs[:], NKT)
                        )
                    nc.sync.dma_start(
                        out=attn_t[h].rearrange("kt p q -> p kt q")[:, :, q0:q0 + SQB],
                        in_=_f32(E[:]),
                    )

        # ---- phase C: output projection out^T = Wo_slice @ ctx^T ----
        with tc.tile_pool(name="osb", bufs=1) as osb, \
             tc.tile_pool(name="ops", bufs=3, space="PSUM") as ops:
            out_sb = osb.tile([128, NC_CHUNKS, S], F32)
            for c in range(NC_CHUNKS):
                for pb in range(S // PBLK):
                    ps = ops.tile([128, PBLK], F32, tag="o")
                    for h in range(HPC):
                        nc.tensor.matmul(
                            ps[:],
                            wo_sb[:, h, c, :],
                            ctx_sb[:, h, pb * PBLK:(pb + 1) * PBLK],
                            start=(h == 0), stop=(h == HPC - 1),
                        )
                    nc.scalar.copy(
                        out=out_sb[:, c, pb * PBLK:(pb + 1) * PBLK], in_=ps[:]
                    )
            nc.sync.dma_start(
                out=out_t[:].rearrange("c p q -> p c q"), in_=out_sb[:]
            )

    nc.compile()
    return nc


def _get_nc():
    global _NC_CACHE
    if _NC_CACHE is None:
        _NC_CACHE = build_nc()
    return _NC_CACHE


def _prep_inputs(query, key, value, mask, W_q, W_k, W_v, W_o):
    """Build the 8 per-core input dicts."""
    SCALE = np.float32(1.0 / np.sqrt(DK))
    xt = {}
    for b in range(B):
        for name, arr in (("qT", query), ("kT", key), ("vT", value)):
            t = np.ascontiguousarray(
                arr[b].T.reshape(NC_CHUNKS, 128, S).transpose(1, 0, 2)
            )
            xt[(name, b)] = t
    pen = [
        np.where(mask[b, 0, 0] == 0, np.float32(-1e9), np.float32(0.0)).astype(
            np.float32
        )
        for b in range(B)
    ]
    in_maps = []
    for core in range(8):
        b, hp = divmod(core, 4)
        h0 = hp * HPC
        sl = slice(h0 * DK, (h0 + HPC) * DK)

        def wslice(W, scale=None):
            ws = W[sl]  # [128, 512] rows = head outputs
            if scale is not None:
                ws = ws * scale
            # [p, c, h, j] = ws[h*64+j, c*128+p]
            return np.ascontiguousarray(
                ws.reshape(HPC, DK, NC_CHUNKS, 128).transpose(3, 2, 0, 1)
            ).astype(np.float32)

        wo_arr = np.ascontiguousarray(
            W_o[:, sl].T.reshape(HPC, DK, NC_CHUNKS, 128).transpose(1, 0, 2, 3)
        ).astype(np.float32)
        auxa = np.empty((2, 1, HPC, S), np.float32)
        auxa[0] = 1.0
        auxa[1, 0, :, :] = pen[b][None, :]
        in_maps.append({
            "qT": xt[("qT", b)],
            "kT": xt[("kT", b)],
            "vT": xt[("vT", b)],
            "wq": wslice(W_q, SCALE),
            "wk": wslice(W_k),
            "wv": wslice(W_v),
            "wo": wo_arr,
            "aux": auxa,
        })
    return in_maps


def kernel(query, key, value, mask, W_q, W_k, W_v, W_o, b_o, _trace=False,
           _trace_kwargs=None):
    query = np.asarray(query, np.float32)
    key = np.asarray(key, np.float32)
    value = np.asarray(value, np.float32)
    mask = np.asarray(mask)
    W_q = np.asarray(W_q, np.float32)
    W_k = np.asarray(W_k, np.float32)
    W_v = np.asarray(W_v, np.float32)
    W_o = np.asarray(W_o, np.float32)
    b_o = np.asarray(b_o, np.float32)

    nc = _get_nc()
    in_maps = _prep_inputs(query, key, value, mask, W_q, W_k, W_v, W_o)
    kw = dict(_trace_kwargs or {})
    res = run_bass_kernel_spmd(nc, in_maps, core_ids=list(range(8)),
                               trace=_trace, **kw)
    kernel.last_result = res

    attn = np.empty((B, H, S, S), np.float32)
    out = np.zeros((B, S, D), np.float32)
    for core in range(8):
        b, hp = divmod(core, 4)
        r = res.results[core]
        for h in range(HPC):
            attn[b, hp * HPC + h] = r["attn_t"][h].reshape(S, S).T
        out[b] += r["out_t"].reshape(D, S).T
    out += b_o
    return out, attn


# revision 10
# speedup vs baseline: 1.5445x; 1.5445x over previous
"""Trainium2 Bass kernel for nn_MultiHeadAttention (B=2, S=4096, D=512, H=8).

Sharding: 8 cores = (2 batches) x (4 head-pairs). Each core computes two
heads' attention for one batch plus its partial output projection.

On-chip orientation is "k-major": S^T[k,q] = K @ Q^T is computed with k on
partitions, so softmax row-sums reduce over the partition axis — fused into
the context matmul via a ones column in V — and the context matmul needs no
transposes at all. The padding mask is folded into the score matmul as an
extra contraction row (lhsT row 64 = penalty, rhs row 64 = 1.0). exp runs on
ScalarE directly out of PSUM. attn is written to HBM fp16 in [k,q] layout and
transposed + upcast on the host during unsharding.

Dtypes: projections fp32r (TF32-like) with fp16 outputs; scores / context /
output-projection matmuls fp16 (fp32 PSUM accumulate).
"""

import os
import sys

sys.path.insert(0, "/opt/trn_rl_repo")

import numpy as np

import concourse.bass as bass
import concourse.mybir as mybir
import concourse.tile as tile
from concourse import bacc
from concourse.bass_utils import run_bass_kernel_spmd
import concourse.bass_utils as bass_utils

# Avoid S3 artifact uploads from the profiling path.
bass_utils.upload_artifacts = lambda tmpdir: f"file://{tmpdir}"

B = 2
S = 4096
D = 512
H = 8
DK = 64
HPC = 2          # heads per core
NCH = 4          # D / 128 contraction chunks
NKT = S // 128   # 32 k-tiles
SQB = 512        # phase-B q strip width
NSTRIP = S // SQB
PBLK = 512       # phase-A projection q block
KTE = 2          # k-tiles per PSUM tile / exp instruction

F32 = mybir.dt.float32
F32R = mybir.dt.float32r
F16 = mybir.dt.float16
PENALTY = -30000.0  # fits fp16; exp(S + PENALTY) == 0 in fp32

_NC_CACHE = None


def _f32(ap):
    return ap.bitcast(F32)


def _bcast_mid(ap, n):
    """Insert a step-0 middle free dim of extent n into a 2D AP."""
    return bass.AP(tensor=ap.tensor, offset=ap.offset, ap=[ap.ap[0], [0, n], ap.ap[1]])


def build_nc():
    nc = bacc.Bacc("TRN2", target_bir_lowering=False, debug=False, num_devices=8)

    qT = nc.dram_tensor("qT", [128, NCH, S], F32R, kind="ExternalInput")
    kT = nc.dram_tensor("kT", [128, NCH, S], F32R, kind="ExternalInput")
    vT = nc.dram_tensor("vT", [128, NCH, S], F32R, kind="ExternalInput")
    wq = nc.dram_tensor("wq", [128, NCH, HPC, DK], F32R, kind="ExternalInput")
    wk = nc.dram_tensor("wk", [128, NCH, HPC, DK], F32R, kind="ExternalInput")
    wv = nc.dram_tensor("wv", [128, NCH, HPC, DK], F32R, kind="ExternalInput")
    wo = nc.dram_tensor("wo", [DK, HPC, NCH, 128], F16, kind="ExternalInput")
    # aux[0] = ones (Q^T row 64), aux[1] = mask penalty (K^T row 64)
    aux = nc.dram_tensor("aux", [2, 1, HPC, S], F16, kind="ExternalInput")

    attn_t = nc.dram_tensor("attn_t", [HPC, NKT, 128, S], F16, kind="ExternalOutput")
    out_t = nc.dram_tensor("out_t", [NCH, 128, S], F32, kind="ExternalOutput")

    from contextlib import ExitStack
    with tile.TileContext(nc) as tc, ExitStack() as stack:
        # ---- persistent SBUF ----
        persist = stack.enter_context(tc.tile_pool(name="persist", bufs=1))
        QTp = persist.tile([65, HPC, S], F16)   # rows 0-63 Q^T, row 64 ones
        KTp = persist.tile([65, HPC, S], F16)   # rows 0-63 K^T, row 64 penalty
        Vsb = persist.tile([128, NKT, HPC, 65], F16)  # cols 0-63 V, col 64 ones
        ctx_sb = persist.tile([DK, HPC, S], F16)
        wq_sb = persist.tile([128, NCH, HPC, DK], F32R)
        wk_sb = persist.tile([128, NCH, HPC, DK], F32R)
        wv_sb = persist.tile([128, NCH, HPC, DK], F32R)
        wo_sb = persist.tile([DK, HPC, NCH, 128], F16)
        ones_row = persist.tile([1, 128], F32R)

        nc.scalar.dma_start(out=wq_sb[:], in_=wq[:])
        nc.scalar.dma_start(out=wk_sb[:], in_=wk[:])
        nc.scalar.dma_start(out=wv_sb[:], in_=wv[:])
        nc.scalar.dma_start(out=wo_sb[:], in_=wo[:])
        nc.scalar.dma_start(out=QTp[64:65, :, :], in_=aux[0, :, :, :])
        nc.scalar.dma_start(out=KTp[64:65, :, :], in_=aux[1, :, :, :])
        nc.vector.memset(_f32(ones_row[:]), 1.0)
        nc.vector.memset(Vsb[:, :, :, 64:65], 1.0)

        # ---- phase A: projections (f32r matmuls, fp16 outputs) ----
        with tc.tile_pool(name="stage", bufs=4) as stage, \
             tc.tile_pool(name="pps", bufs=3, space="PSUM") as pps:
            for (src, w_sb, dstp) in ((qT, wq_sb, QTp), (kT, wk_sb, KTp)):
                chunks = []
                for c in range(NCH):
                    t = stage.tile([128, S], F32R, tag="stage")
                    nc.scalar.dma_start(out=t[:], in_=src[:, c, :])
                    chunks.append(t)
                for h in range(HPC):
                    for pb in range(S // PBLK):
                        ps = pps.tile([DK, PBLK], F32, tag="proj")
                        for c in range(NCH):
                            nc.tensor.matmul(
                                ps[:],
                                w_sb[:, c, h, :],
                                chunks[c][:, pb * PBLK:(pb + 1) * PBLK],
                                start=(c == 0), stop=(c == NCH - 1),
                            )
                        nc.vector.tensor_copy(
                            out=dstp[0:DK, h, pb * PBLK:(pb + 1) * PBLK], in_=ps[:]
                        )
            # V = value @ Wv^T, built [k, dv] with k on partitions
            chunks = []
            for c in range(NCH):
                t = stage.tile([128, S], F32R, tag="stage")
                nc.scalar.dma_start(out=t[:], in_=vT[:, c, :])
                chunks.append(t)
            for kt in range(NKT):
                ps = pps.tile([128, HPC * DK], F32, tag="vproj")
                for c in range(NCH):
                    nc.tensor.matmul(
                        ps[:],
                        chunks[c][:, kt * 128:(kt + 1) * 128],
                        wv_sb[:, c, :, :],
                        start=(c == 0), stop=(c == NCH - 1),
                    )
                for h in range(HPC):
                    nc.vector.tensor_copy(
                        out=Vsb[:, kt, h, 0:DK], in_=ps[:, h * DK:(h + 1) * DK]
                    )

        # ---- phase B: scores, softmax, context (fp16 matmuls) ----
        with tc.tile_pool(name="epool", bufs=2) as epool, \
             tc.tile_pool(name="small", bufs=3) as small, \
             tc.tile_pool(name="invp", bufs=2) as invp, \
             tc.tile_pool(name="sps", bufs=2, space="PSUM") as sps, \
             tc.tile_pool(name="cps", bufs=2, space="PSUM") as cps, \
             tc.tile_pool(name="ips", bufs=2, space="PSUM") as ips:
            for h in range(HPC):
                for sb in range(NSTRIP):
                    q0 = sb * SQB
                    E = epool.tile([128, NKT, SQB], F16, tag="E")
                    cx = cps.tile([65, SQB], F32, tag="ctx")
                    for kte in range(NKT // KTE):
                        sp = sps.tile([128, KTE, SQB], F32, tag="s")
                        for j in range(KTE):
                            kt = kte * KTE + j
                            nc.tensor.matmul(
                                sp[:, j, :],
                                KTp[:, h, kt * 128:(kt + 1) * 128],
                                QTp[:, h, q0:q0 + SQB],
                                start=True, stop=True,
                            )
                        nc.scalar.activation(
                            out=E[:, kte * KTE:(kte + 1) * KTE, :],
                            in_=sp[:],
                            func=mybir.ActivationFunctionType.Exp,
                        )
                        for j in range(KTE):
                            kt = kte * KTE + j
                            nc.tensor.matmul(
                                cx[:],
                                Vsb[:, kt, h, 0:65],
                                E[:, kt, :],
                                start=(kt == 0), stop=(kt == NKT - 1),
                            )
                    inv = small.tile([1, SQB], F32R, tag="inv")
                    with nc.allow_low_precision(reason="f32r inv for f32r matmul"):
                        nc.vector.reciprocal(out=inv[:], in_=cx[64:65, :])
                    ib = ips.tile([128, SQB], F32, tag="ib")
                    nc.tensor.matmul(ib[:], ones_row[:], inv[:], start=True, stop=True)
                    ibs = invp.tile([128, SQB], F16, tag="ibs")
                    nc.vector.tensor_copy(out=ibs[:], in_=ib[:])
                    with nc.allow_low_precision(reason="fp16 P/ctx tiles"):
                        nc.vector.tensor_mul(
                            ctx_sb[:, h, q0:q0 + SQB], cx[0:DK, :], ibs[0:DK, :]
                        )
                        nc.vector.tensor_mul(
                            E[:], E[:], _bcast_mid(ibs[:], NKT)
                        )
                    nc.sync.dma_start(
                        out=attn_t[h].rearrange("kt p q -> p kt q")[:, :, q0:q0 + SQB],
                        in_=E[:],
                    )

        # ---- phase C: output projection out^T = Wo_slice @ ctx^T ----
        with tc.tile_pool(name="osb", bufs=1) as osb, \
             tc.tile_pool(name="ops", bufs=3, space="PSUM") as ops:
            out_sb = osb.tile([128, NCH, S], F32)
            for c in range(NCH):
                for pb in range(S // PBLK):
                    ps = ops.tile([128, PBLK], F32, tag="o")
                    for h in range(HPC):
                        nc.tensor.matmul(
                            ps[:],
                            wo_sb[:, h, c, :],
                            ctx_sb[:, h, pb * PBLK:(pb + 1) * PBLK],
                            start=(h == 0), stop=(h == HPC - 1),
                        )
                    nc.scalar.copy(
                        out=out_sb[:, c, pb * PBLK:(pb + 1) * PBLK], in_=ps[:]
                    )
            nc.sync.dma_start(
                out=out_t[:].rearrange("c p q -> p c q"), in_=out_sb[:]
            )

    nc.compile()
    return nc


def _get_nc():
    global _NC_CACHE
    if _NC_CACHE is None:
        _NC_CACHE = build_nc()
    return _NC_CACHE


def _prep_inputs(query, key, value, mask, W_q, W_k, W_v, W_o):
    """Build the 8 per-core input dicts."""
    SCALE = np.float32(1.0 / np.sqrt(DK))
    xt = {}
    for b in range(B):
        for name, arr in (("qT", query), ("kT", key), ("vT", value)):
            t = np.ascontiguousarray(
                arr[b].T.reshape(NCH, 128, S).transpose(1, 0, 2)
            )
            xt[(name, b)] = t
    pen = [
        np.where(mask[b, 0, 0] == 0, np.float32(PENALTY), np.float32(0.0)).astype(
            np.float16
        )
        for b in range(B)
    ]
    in_maps = []
    for core in range(8):
        b, hp = divmod(core, 4)
        h0 = hp * HPC
        sl = slice(h0 * DK, (h0 + HPC) * DK)

        def wslice(W, scale=None):
            ws = W[sl]  # [128, 512] rows = head outputs
            if scale is not None:
                ws = ws * scale
            # [p, c, h, j] = ws[h*64+j, c*128+p]
            return np.ascontiguousarray(
                ws.reshape(HPC, DK, NCH, 128).transpose(3, 2, 0, 1)
            ).astype(np.float32)

        wo_arr = np.ascontiguousarray(
            W_o[:, sl].T.reshape(HPC, DK, NCH, 128).transpose(1, 0, 2, 3)
        ).astype(np.float16)
        auxa = np.empty((2, 1, HPC, S), np.float16)
        auxa[0] = np.float16(1.0)
        auxa[1, 0, :, :] = pen[b][None, :]
        in_maps.append({
            "qT": xt[("qT", b)],
            "kT": xt[("kT", b)],
            "vT": xt[("vT", b)],
            "wq": wslice(W_q, SCALE),
            "wk": wslice(W_k),
            "wv": wslice(W_v),
            "wo": wo_arr,
            "aux": auxa,
        })
    return in_maps


def kernel(query, key, value, mask, W_q, W_k, W_v, W_o, b_o, _trace=False,
           _trace_kwargs=None):
    query = np.asarray(query, np.float32)
    key = np.asarray(key, np.float32)
    value = np.asarray(value, np.float32)
    mask = np.asarray(mask)
    W_q = np.asarray(W_q, np.float32)
    W_k = np.asarray(W_k, np.float32)
    W_v = np.asarray(W_v, np.float32)
    W_o = np.asarray(W_o, np.float32)
    b_o = np.asarray(b_o, np.float32)

    nc = _get_nc()
    in_maps = _prep_inputs(query, key, value, mask, W_q, W_k, W_v, W_o)
    kw = dict(_trace_kwargs or {})
    res = run_bass_kernel_spmd(nc, in_maps, core_ids=list(range(8)),
                               trace=_trace, **kw)
    kernel.last_result = res

    attn = np.empty((B, H, S, S), np.float32)
    out = np.zeros((B, S, D), np.float32)
    for core in range(8):
        b, hp = divmod(core, 4)
        r = res.results[core]
        for h in range(HPC):
            attn[b, hp * HPC + h] = r["attn_t"][h].reshape(S, S).T
        out[b] += r["out_t"].reshape(D, S).T
    out += b_o
    return out, attn


# revision 11
# speedup vs baseline: 1.6629x; 1.0766x over previous
"""Trainium2 Bass kernel for nn_MultiHeadAttention (B=2, S=4096, D=512, H=8).

Sharding: 8 cores = (2 batches) x (4 head-pairs). Each core computes two
heads' attention for one batch plus its partial output projection.

On-chip orientation is "k-major": S^T[k,q] = K @ Q^T is computed with k on
partitions, so softmax row-sums reduce over the partition axis — fused into
the context matmul via a ones column in V — and the context matmul needs no
transposes at all. The padding mask is folded into the score matmul as an
extra contraction row (lhsT row 64 = penalty, rhs row 64 = 1.0). exp runs on
ScalarE directly out of PSUM; the 1/rowsum vector is broadcast across
partitions by GpSimd so TensorE never stalls on strip epilogues. The output
projection is interleaved into the strip loop one strip behind. attn is
written to HBM fp16 in [k,q] layout and transposed + upcast on the host
during unsharding.

All matmuls are fp16 with fp32 PSUM accumulation.
"""

import os
import sys

sys.path.insert(0, "/opt/trn_rl_repo")

import numpy as np

import concourse.bass as bass
import concourse.mybir as mybir
import concourse.tile as tile
from concourse import bacc
from concourse.bass_utils import run_bass_kernel_spmd
import concourse.bass_utils as bass_utils

# Avoid S3 artifact uploads from the profiling path.
bass_utils.upload_artifacts = lambda tmpdir: f"file://{tmpdir}"

B = 2
S = 4096
D = 512
H = 8
DK = 64
HPC = 2          # heads per core
NCH = 4          # D / 128 contraction chunks
NKT = S // 128   # 32 k-tiles
SQB = 512        # phase-B q strip width
NSTRIP = S // SQB
PBLK = 512       # phase-A projection q block
KTE = 2          # k-tiles per PSUM tile / exp instruction

F32 = mybir.dt.float32
F32R = mybir.dt.float32r
F16 = mybir.dt.float16
PENALTY = -30000.0  # fits fp16; exp(S + PENALTY) == 0 in fp32

_NC_CACHE = None


def _f32(ap):
    return ap.bitcast(F32)


def _bcast_mid(ap, n):
    """Insert a step-0 middle free dim of extent n into a 2D AP."""
    return bass.AP(tensor=ap.tensor, offset=ap.offset, ap=[ap.ap[0], [0, n], ap.ap[1]])


def build_nc():
    nc = bacc.Bacc("TRN2", target_bir_lowering=False, debug=False, num_devices=8)

    qT = nc.dram_tensor("qT", [128, NCH, S], F16, kind="ExternalInput")
    kT = nc.dram_tensor("kT", [128, NCH, S], F16, kind="ExternalInput")
    vT = nc.dram_tensor("vT", [128, NCH, S], F16, kind="ExternalInput")
    wq = nc.dram_tensor("wq", [128, NCH, HPC, DK], F16, kind="ExternalInput")
    wk = nc.dram_tensor("wk", [128, NCH, HPC, DK], F16, kind="ExternalInput")
    wv = nc.dram_tensor("wv", [128, NCH, HPC, DK], F16, kind="ExternalInput")
    wo = nc.dram_tensor("wo", [DK, HPC, NCH, 128], F16, kind="ExternalInput")
    # aux[0] = ones (Q^T row 64), aux[1] = mask penalty (K^T row 64)
    aux = nc.dram_tensor("aux", [2, 1, HPC, S], F16, kind="ExternalInput")

    attn_t = nc.dram_tensor("attn_t", [HPC, NKT, 128, S], F16, kind="ExternalOutput")
    out_t = nc.dram_tensor("out_t", [NCH, 128, S], F32, kind="ExternalOutput")

    from contextlib import ExitStack
    with tile.TileContext(nc) as tc, ExitStack() as stack:
        # ---- persistent SBUF ----
        persist = stack.enter_context(tc.tile_pool(name="persist", bufs=1))
        QTp = persist.tile([65, HPC, S], F16)   # rows 0-63 Q^T, row 64 ones
        KTp = persist.tile([65, HPC, S], F16)   # rows 0-63 K^T, row 64 penalty
        Vsb = persist.tile([128, NKT, HPC, 65], F16)  # cols 0-63 V, col 64 ones
        ctx_sb = persist.tile([DK, HPC, S], F16)
        wq_sb = persist.tile([128, NCH, HPC, DK], F16)
        wk_sb = persist.tile([128, NCH, HPC, DK], F16)
        wv_sb = persist.tile([128, NCH, HPC, DK], F16)
        wo_sb = persist.tile([DK, HPC, NCH, 128], F16)

        nc.scalar.dma_start(out=wq_sb[:], in_=wq[:])
        nc.scalar.dma_start(out=wk_sb[:], in_=wk[:])
        nc.scalar.dma_start(out=wv_sb[:], in_=wv[:])
        nc.scalar.dma_start(out=wo_sb[:], in_=wo[:])
        nc.scalar.dma_start(out=QTp[64:65, :, :], in_=aux[0, :, :, :])
        nc.scalar.dma_start(out=KTp[64:65, :, :], in_=aux[1, :, :, :])
        nc.vector.memset(Vsb[:, :, :, 64:65], 1.0)

        # ---- phase A: projections (fp16 matmuls) ----
        with tc.tile_pool(name="stage", bufs=4) as stage, \
             tc.tile_pool(name="pps", bufs=3, space="PSUM") as pps:
            for (src, w_sb, dstp) in ((qT, wq_sb, QTp), (kT, wk_sb, KTp)):
                chunks = []
                for c in range(NCH):
                    t = stage.tile([128, S], F16, tag="stage")
                    nc.scalar.dma_start(out=t[:], in_=src[:, c, :])
                    chunks.append(t)
                for h in range(HPC):
                    for pb in range(S // PBLK):
                        ps = pps.tile([DK, PBLK], F32, tag="proj")
                        for c in range(NCH):
                            nc.tensor.matmul(
                                ps[:],
                                w_sb[:, c, h, :],
                                chunks[c][:, pb * PBLK:(pb + 1) * PBLK],
                                start=(c == 0), stop=(c == NCH - 1),
                            )
                        nc.vector.tensor_copy(
                            out=dstp[0:DK, h, pb * PBLK:(pb + 1) * PBLK], in_=ps[:]
                        )
            # V = value @ Wv^T, built [k, dv] with k on partitions
            chunks = []
            for c in range(NCH):
                t = stage.tile([128, S], F16, tag="stage")
                nc.scalar.dma_start(out=t[:], in_=vT[:, c, :])
                chunks.append(t)
            for kt in range(NKT):
                ps = pps.tile([128, HPC * DK], F32, tag="vproj")
                for c in range(NCH):
                    nc.tensor.matmul(
                        ps[:],
                        chunks[c][:, kt * 128:(kt + 1) * 128],
                        wv_sb[:, c, :, :],
                        start=(c == 0), stop=(c == NCH - 1),
                    )
                for h in range(HPC):
                    nc.vector.tensor_copy(
                        out=Vsb[:, kt, h, 0:DK], in_=ps[:, h * DK:(h + 1) * DK]
                    )

        # ---- phase B: scores, softmax, context, interleaved out-proj ----
        with tc.tile_pool(name="epool", bufs=2) as epool, \
             tc.tile_pool(name="small", bufs=3) as small, \
             tc.tile_pool(name="invp", bufs=2) as invp, \
             tc.tile_pool(name="oblk", bufs=2) as oblk, \
             tc.tile_pool(name="sps", bufs=2, space="PSUM") as sps, \
             tc.tile_pool(name="cps", bufs=2, space="PSUM") as cps, \
             tc.tile_pool(name="ops", bufs=2, space="PSUM") as ops:

            def emit_out_proj(q0):
                """Output projection for q-block q0 (both heads' ctx ready)."""
                ob = oblk.tile([128, NCH, PBLK], F32, tag="ob")
                for c in range(NCH):
                    ps = ops.tile([128, PBLK], F32, tag="o")
                    for h in range(HPC):
                        nc.tensor.matmul(
                            ps[:],
                            wo_sb[:, h, c, :],
                            ctx_sb[:, h, q0:q0 + PBLK],
                            start=(h == 0), stop=(h == HPC - 1),
                        )
                    nc.scalar.copy(out=ob[:, c, :], in_=ps[:])
                nc.sync.dma_start(
                    out=out_t[:].rearrange("c p q -> p c q")[:, :, q0:q0 + PBLK],
                    in_=ob[:],
                )

            for sb in range(NSTRIP):
                q0 = sb * SQB
                for h in range(HPC):
                    E = epool.tile([128, NKT, SQB], F16, tag="E")
                    cx = cps.tile([65, SQB], F32, tag="ctx")
                    for kte in range(NKT // KTE):
                        sp = sps.tile([128, KTE, SQB], F32, tag="s")
                        for j in range(KTE):
                            kt = kte * KTE + j
                            nc.tensor.matmul(
                                sp[:, j, :],
                                KTp[:, h, kt * 128:(kt + 1) * 128],
                                QTp[:, h, q0:q0 + SQB],
                                start=True, stop=True,
                            )
                        nc.scalar.activation(
                            out=E[:, kte * KTE:(kte + 1) * KTE, :],
                            in_=sp[:],
                            func=mybir.ActivationFunctionType.Exp,
                        )
                        for j in range(KTE):
                            kt = kte * KTE + j
                            nc.tensor.matmul(
                                cx[:],
                                Vsb[:, kt, h, 0:65],
                                E[:, kt, :],
                                start=(kt == 0), stop=(kt == NKT - 1),
                            )
                    # strip epilogue: no TensorE work in here
                    inv = small.tile([1, SQB], F16, tag="inv")
                    with nc.allow_low_precision(reason="fp16 softmax normalize"):
                        nc.vector.reciprocal(out=inv[:], in_=cx[64:65, :])
                    ibs = invp.tile([128, SQB], F16, tag="ibs")
                    nc.gpsimd.partition_broadcast(ibs[:], inv[:])
                    with nc.allow_low_precision(reason="fp16 P/ctx tiles"):
                        nc.vector.tensor_mul(
                            ctx_sb[:, h, q0:q0 + SQB], cx[0:DK, :], ibs[0:DK, :]
                        )
                        nc.vector.tensor_mul(
                            E[:], E[:], _bcast_mid(ibs[:], NKT)
                        )
                    nc.sync.dma_start(
                        out=attn_t[h].rearrange("kt p q -> p kt q")[:, :, q0:q0 + SQB],
                        in_=E[:],
                    )
                if sb > 0:
                    emit_out_proj((sb - 1) * SQB)
            emit_out_proj((NSTRIP - 1) * SQB)

    nc.compile()
    return nc


def _get_nc():
    global _NC_CACHE
    if _NC_CACHE is None:
        _NC_CACHE = build_nc()
    return _NC_CACHE


def _prep_inputs(query, key, value, mask, W_q, W_k, W_v, W_o):
    """Build the 8 per-core input dicts."""
    SCALE = np.float32(1.0 / np.sqrt(DK))
    xt = {}
    for b in range(B):
        for name, arr in (("qT", query), ("kT", key), ("vT", value)):
            t = np.ascontiguousarray(
                arr[b].T.reshape(NCH, 128, S).transpose(1, 0, 2).astype(np.float16)
            )
            xt[(name, b)] = t
    pen = [
        np.where(mask[b, 0, 0] == 0, np.float32(PENALTY), np.float32(0.0)).astype(
            np.float16
        )
        for b in range(B)
    ]
    in_maps = []
    for core in range(8):
        b, hp = divmod(core, 4)
        h0 = hp * HPC
        sl = slice(h0 * DK, (h0 + HPC) * DK)

        def wslice(W, scale=None):
            ws = W[sl]  # [128, 512] rows = head outputs
            if scale is not None:
                ws = ws * scale
            # [p, c, h, j] = ws[h*64+j, c*128+p]
            return np.ascontiguousarray(
                ws.reshape(HPC, DK, NCH, 128).transpose(3, 2, 0, 1)
            ).astype(np.float16)

        wo_arr = np.ascontiguousarray(
            W_o[:, sl].T.reshape(HPC, DK, NCH, 128).transpose(1, 0, 2, 3)
        ).astype(np.float16)
        auxa = np.empty((2, 1, HPC, S), np.float16)
        auxa[0] = np.float16(1.0)
        auxa[1, 0, :, :] = pen[b][None, :]
        in_maps.append({
            "qT": xt[("qT", b)],
            "kT": xt[("kT", b)],
            "vT": xt[("vT", b)],
            "wq": wslice(W_q, SCALE),
            "wk": wslice(W_k),
            "wv": wslice(W_v),
            "wo": wo_arr,
            "aux": auxa,
        })
    return in_maps


def kernel(query, key, value, mask, W_q, W_k, W_v, W_o, b_o, _trace=False,
           _trace_kwargs=None):
    query = np.asarray(query, np.float32)
    key = np.asarray(key, np.float32)
    value = np.asarray(value, np.float32)
    mask = np.asarray(mask)
    W_q = np.asarray(W_q, np.float32)
    W_k = np.asarray(W_k, np.float32)
    W_v = np.asarray(W_v, np.float32)
    W_o = np.asarray(W_o, np.float32)
    b_o = np.asarray(b_o, np.float32)

    nc = _get_nc()
    in_maps = _prep_inputs(query, key, value, mask, W_q, W_k, W_v, W_o)
    kw = dict(_trace_kwargs or {})
    res = run_bass_kernel_spmd(nc, in_maps, core_ids=list(range(8)),
                               trace=_trace, **kw)
    kernel.last_result = res

    attn = np.empty((B, H, S, S), np.float32)
    out = np.zeros((B, S, D), np.float32)
    for core in range(8):
        b, hp = divmod(core, 4)
        r = res.results[core]
        for h in range(HPC):
            attn[b, hp * HPC + h] = r["attn_t"][h].reshape(S, S).T
        out[b] += r["out_t"].reshape(D, S).T
    out += b_o
    return out, attn


# revision 14
# speedup vs baseline: 1.7238x; 1.0366x over previous
"""Trainium2 Bass kernel for nn_MultiHeadAttention (B=2, S=4096, D=512, H=8).

Sharding: 8 cores = (2 batches) x (4 head-pairs). Each core computes two
heads' attention for one batch plus its partial output projection.

On-chip orientation is "k-major": S^T[k,q] = K @ Q^T is computed with k on
partitions, so softmax row-sums reduce over the partition axis — fused into
the context matmul via a ones column in V — and the context matmul needs no
transposes at all. The padding mask is folded into the score matmul as an
extra contraction row (lhsT row 64 = penalty, rhs row 64 = 1.0). exp runs on
ScalarE directly out of PSUM; the 1/rowsum vector is broadcast across
partitions by GpSimd so TensorE never stalls on strip epilogues. The output
projection is interleaved into the strip loop one strip behind. attn is
written to HBM fp16 in [k,q] layout and transposed + upcast on the host
during unsharding.

All matmuls are fp16 with fp32 PSUM accumulation.
"""

import os
import sys

sys.path.insert(0, "/opt/trn_rl_repo")

import numpy as np

import concourse.bass as bass
import concourse.mybir as mybir
import concourse.tile as tile
from concourse import bacc
from concourse.bass_utils import run_bass_kernel_spmd
import concourse.bass_utils as bass_utils

# Avoid S3 artifact uploads from the profiling path.
bass_utils.upload_artifacts = lambda tmpdir: f"file://{tmpdir}"

B = 2
S = 4096
D = 512
H = 8
DK = 64
HPC = 2          # heads per core
NCH = 4          # D / 128 contraction chunks
NKT = S // 128   # 32 k-tiles
SQB = 512        # phase-B q strip width
NSTRIP = S // SQB
PBLK = 512       # phase-A projection q block
KTE = 2          # k-tiles per PSUM tile / exp instruction

F32 = mybir.dt.float32
F32R = mybir.dt.float32r
F16 = mybir.dt.float16
PENALTY = -30000.0  # fits fp16; exp(S + PENALTY) == 0 in fp32

_NC_CACHE = None


def _f32(ap):
    return ap.bitcast(F32)


def _bcast_mid(ap, n):
    """Insert a step-0 middle free dim of extent n into a 2D AP."""
    return bass.AP(tensor=ap.tensor, offset=ap.offset, ap=[ap.ap[0], [0, n], ap.ap[1]])


def build_nc():
    nc = bacc.Bacc("TRN2", target_bir_lowering=False, debug=False, num_devices=8)

    qT = nc.dram_tensor("qT", [128, NCH, S], F16, kind="ExternalInput")
    kT = nc.dram_tensor("kT", [128, NCH, S], F16, kind="ExternalInput")
    vT = nc.dram_tensor("vT", [128, NCH, S], F16, kind="ExternalInput")
    wq = nc.dram_tensor("wq", [128, NCH, HPC, DK], F16, kind="ExternalInput")
    wk = nc.dram_tensor("wk", [128, NCH, HPC, DK], F16, kind="ExternalInput")
    wv = nc.dram_tensor("wv", [128, NCH, HPC, DK], F16, kind="ExternalInput")
    wo = nc.dram_tensor("wo", [DK, HPC, NCH, 128], F16, kind="ExternalInput")
    # aux[0] = ones (Q^T row 64), aux[1] = mask penalty (K^T row 64)
    aux = nc.dram_tensor("aux", [2, 1, HPC, S], F16, kind="ExternalInput")

    attn_t = nc.dram_tensor("attn_t", [HPC, NKT, 128, S], F16, kind="ExternalOutput")
    out_t = nc.dram_tensor("out_t", [NCH, 128, S], F32, kind="ExternalOutput")

    from contextlib import ExitStack
    with tile.TileContext(nc) as tc, ExitStack() as stack:
        # ---- persistent SBUF ----
        persist = stack.enter_context(tc.tile_pool(name="persist", bufs=1))
        QTp = persist.tile([128, HPC, S], F16)  # rows 0-63 Q^T, row 64 ones, rest 0
        KTp = persist.tile([128, HPC, S], F16)  # rows 0-63 K^T, row 64 penalty
        Vsb = persist.tile([128, NKT, HPC, 65], F16)  # cols 0-63 V, col 64 ones
        ctx_sb = persist.tile([DK, HPC, S], F16)
        wq_sb = persist.tile([128, NCH, HPC, DK], F16)
        wk_sb = persist.tile([128, NCH, HPC, DK], F16)
        wv_sb = persist.tile([128, NCH, HPC, DK], F16)
        wo_sb = persist.tile([DK, HPC, NCH, 128], F16)

        nc.scalar.dma_start(out=wq_sb[:], in_=wq[:])
        nc.scalar.dma_start(out=wk_sb[:], in_=wk[:])
        nc.scalar.dma_start(out=wv_sb[:], in_=wv[:])
        nc.scalar.dma_start(out=wo_sb[:], in_=wo[:])
        nc.vector.memset(Vsb[:, :, :, 64:65], 1.0)
        # zero the padding rows so the 128-deep contraction is exact (and the
        # full-depth weight load path can engage); row 64 is then overwritten
        # by the aux DMAs below
        nc.vector.memset(QTp[64:128, :, :], 0.0)
        nc.vector.memset(KTp[64:128, :, :], 0.0)
        nc.scalar.dma_start(out=QTp[64:65, :, :], in_=aux[0, :, :, :])
        nc.scalar.dma_start(out=KTp[64:65, :, :], in_=aux[1, :, :, :])

        # ---- phase A: projections (fp16 matmuls) ----
        with tc.tile_pool(name="stage", bufs=4) as stage, \
             tc.tile_pool(name="pps", bufs=3, space="PSUM") as pps:
            for (src, w_sb, dstp) in ((qT, wq_sb, QTp), (kT, wk_sb, KTp)):
                chunks = []
                for c in range(NCH):
                    t = stage.tile([128, S], F16, tag="stage")
                    nc.scalar.dma_start(out=t[:], in_=src[:, c, :])
                    chunks.append(t)
                for h in range(HPC):
                    for pb in range(S // PBLK):
                        ps = pps.tile([DK, PBLK], F32, tag="proj")
                        for c in range(NCH):
                            nc.tensor.matmul(
                                ps[:],
                                w_sb[:, c, h, :],
                                chunks[c][:, pb * PBLK:(pb + 1) * PBLK],
                                start=(c == 0), stop=(c == NCH - 1),
                            )
                        nc.vector.tensor_copy(
                            out=dstp[0:DK, h, pb * PBLK:(pb + 1) * PBLK], in_=ps[:]
                        )
            # V = value @ Wv^T, built [k, dv] with k on partitions
            chunks = []
            for c in range(NCH):
                t = stage.tile([128, S], F16, tag="stage")
                nc.scalar.dma_start(out=t[:], in_=vT[:, c, :])
                chunks.append(t)
            for kt in range(NKT):
                ps = pps.tile([128, HPC * DK], F32, tag="vproj")
                for c in range(NCH):
                    nc.tensor.matmul(
                        ps[:],
                        chunks[c][:, kt * 128:(kt + 1) * 128],
                        wv_sb[:, c, :, :],
                        start=(c == 0), stop=(c == NCH - 1),
                    )
                for h in range(HPC):
                    nc.vector.tensor_copy(
                        out=Vsb[:, kt, h, 0:DK], in_=ps[:, h * DK:(h + 1) * DK]
                    )

        # ---- phase B: scores, softmax, context, interleaved out-proj ----
        with tc.tile_pool(name="epool", bufs=2) as epool, \
             tc.tile_pool(name="small", bufs=3) as small, \
             tc.tile_pool(name="invp", bufs=2) as invp, \
             tc.tile_pool(name="oblk", bufs=2) as oblk, \
             tc.tile_pool(name="sps", bufs=2, space="PSUM") as sps, \
             tc.tile_pool(name="cps", bufs=2, space="PSUM") as cps, \
             tc.tile_pool(name="ops", bufs=2, space="PSUM") as ops:

            def emit_out_proj(q0):
                """Output projection for q-block q0 (both heads' ctx ready)."""
                ob = oblk.tile([128, NCH, PBLK], F32, tag="ob")
                for c in range(NCH):
                    ps = ops.tile([128, PBLK], F32, tag="o")
                    for h in range(HPC):
                        nc.tensor.matmul(
                            ps[:],
                            wo_sb[:, h, c, :],
                            ctx_sb[:, h, q0:q0 + PBLK],
                            start=(h == 0), stop=(h == HPC - 1),
                        )
                    nc.scalar.copy(out=ob[:, c, :], in_=ps[:])
                nc.sync.dma_start(
                    out=out_t[:].rearrange("c p q -> p c q")[:, :, q0:q0 + PBLK],
                    in_=ob[:],
                )

            for sb in range(NSTRIP):
                q0 = sb * SQB
                for h in range(HPC):
                    E = epool.tile([128, NKT, SQB], F16, tag="E")
                    cx = cps.tile([65, SQB], F32, tag="ctx")
                    for kte in range(NKT // KTE):
                        sp = sps.tile([128, KTE, SQB], F32, tag="s")
                        for j in range(KTE):
                            kt = kte * KTE + j
                            nc.tensor.matmul(
                                sp[:, j, :],
                                KTp[:, h, kt * 128:(kt + 1) * 128],
                                QTp[:, h, q0:q0 + SQB],
                                start=True, stop=True,
                            )
                        nc.scalar.activation(
                            out=E[:, kte * KTE:(kte + 1) * KTE, :],
                            in_=sp[:],
                            func=mybir.ActivationFunctionType.Exp,
                        )
                        for j in range(KTE):
                            kt = kte * KTE + j
                            nc.tensor.matmul(
                                cx[:],
                                Vsb[:, kt, h, 0:65],
                                E[:, kt, :],
                                start=(kt == 0), stop=(kt == NKT - 1),
                            )
                    # strip epilogue: no TensorE work in here
                    inv = small.tile([1, SQB], F16, tag="inv")
                    with nc.allow_low_precision(reason="fp16 softmax normalize"):
                        nc.vector.reciprocal(out=inv[:], in_=cx[64:65, :])
                    ibs = invp.tile([128, SQB], F16, tag="ibs")
                    nc.gpsimd.partition_broadcast(ibs[:], inv[:])
                    with nc.allow_low_precision(reason="fp16 P/ctx tiles"):
                        nc.vector.tensor_mul(
                            ctx_sb[:, h, q0:q0 + SQB], cx[0:DK, :], ibs[0:DK, :]
                        )
                        nc.vector.tensor_mul(
                            E[:], E[:], _bcast_mid(ibs[:], NKT)
                        )
                    nc.sync.dma_start(
                        out=attn_t[h].rearrange("kt p q -> p kt q")[:, :, q0:q0 + SQB],
                        in_=E[:],
                    )
                if sb > 0:
                    emit_out_proj((sb - 1) * SQB)
            emit_out_proj((NSTRIP - 1) * SQB)

    nc.compile()
    return nc


def _get_nc():
    global _NC_CACHE
    if _NC_CACHE is None:
        _NC_CACHE = build_nc()
    return _NC_CACHE


def _prep_inputs(query, key, value, mask, W_q, W_k, W_v, W_o):
    """Build the 8 per-core input dicts."""
    SCALE = np.float32(1.0 / np.sqrt(DK))
    xt = {}
    for b in range(B):
        for name, arr in (("qT", query), ("kT", key), ("vT", value)):
            t = np.ascontiguousarray(
                arr[b].T.reshape(NCH, 128, S).transpose(1, 0, 2).astype(np.float16)
            )
            xt[(name, b)] = t
    pen = [
        np.where(mask[b, 0, 0] == 0, np.float32(PENALTY), np.float32(0.0)).astype(
            np.float16
        )
        for b in range(B)
    ]
    in_maps = []
    for core in range(8):
        b, hp = divmod(core, 4)
        h0 = hp * HPC
        sl = slice(h0 * DK, (h0 + HPC) * DK)

        def wslice(W, scale=None):
            ws = W[sl]  # [128, 512] rows = head outputs
            if scale is not None:
                ws = ws * scale
            # [p, c, h, j] = ws[h*64+j, c*128+p]
            return np.ascontiguousarray(
                ws.reshape(HPC, DK, NCH, 128).transpose(3, 2, 0, 1)
            ).astype(np.float16)

        wo_arr = np.ascontiguousarray(
            W_o[:, sl].T.reshape(HPC, DK, NCH, 128).transpose(1, 0, 2, 3)
        ).astype(np.float16)
        auxa = np.empty((2, 1, HPC, S), np.float16)
        auxa[0] = np.float16(1.0)
        auxa[1, 0, :, :] = pen[b][None, :]
        in_maps.append({
            "qT": xt[("qT", b)],
            "kT": xt[("kT", b)],
            "vT": xt[("vT", b)],
            "wq": wslice(W_q, SCALE),
            "wk": wslice(W_k),
            "wv": wslice(W_v),
            "wo": wo_arr,
            "aux": auxa,
        })
    return in_maps


def kernel(query, key, value, mask, W_q, W_k, W_v, W_o, b_o, _trace=False,
           _trace_kwargs=None):
    query = np.asarray(query, np.float32)
    key = np.asarray(key, np.float32)
    value = np.asarray(value, np.float32)
    mask = np.asarray(mask)
    W_q = np.asarray(W_q, np.float32)
    W_k = np.asarray(W_k, np.float32)
    W_v = np.asarray(W_v, np.float32)
    W_o = np.asarray(W_o, np.float32)
    b_o = np.asarray(b_o, np.float32)

    nc = _get_nc()
    in_maps = _prep_inputs(query, key, value, mask, W_q, W_k, W_v, W_o)
    kw = dict(_trace_kwargs or {})
    res = run_bass_kernel_spmd(nc, in_maps, core_ids=list(range(8)),
                               trace=_trace, **kw)
    kernel.last_result = res

    attn = np.empty((B, H, S, S), np.float32)
    out = np.zeros((B, S, D), np.float32)
    for core in range(8):
        b, hp = divmod(core, 4)
        r = res.results[core]
        for h in range(HPC):
            attn[b, hp * HPC + h] = r["attn_t"][h].reshape(S, S).T
        out[b] += r["out_t"].reshape(D, S).T
    out += b_o
    return out, attn


# revision 16
# speedup vs baseline: 1.7729x; 1.0285x over previous
"""Trainium2 Bass kernel for nn_MultiHeadAttention (B=2, S=4096, D=512, H=8).

Sharding: 8 cores = (2 batches) x (4 head-pairs). Each core computes two
heads' attention for one batch plus its partial output projection.

On-chip orientation is "k-major": S^T[k,q] = K @ Q^T is computed with k on
partitions, so softmax row-sums reduce over the partition axis — fused into
the context matmul via a ones column in V — and the context matmul needs no
transposes at all. The padding mask is folded into the score matmul as an
extra contraction row (lhsT row 64 = penalty, rhs row 64 = 1.0). exp runs on
ScalarE directly out of PSUM; the 1/rowsum vector is broadcast across
partitions by GpSimd so TensorE never stalls on strip epilogues. The output
projection is interleaved into the strip loop one strip behind. attn is
written to HBM fp16 in [k,q] layout and transposed + upcast on the host
during unsharding.

All matmuls are fp16 with fp32 PSUM accumulation.
"""

import os
import sys

sys.path.insert(0, "/opt/trn_rl_repo")

import numpy as np

import concourse.bass as bass
import concourse.mybir as mybir
import concourse.tile as tile
from concourse import bacc
from concourse.bass_utils import run_bass_kernel_spmd
import concourse.bass_utils as bass_utils

# Avoid S3 artifact uploads from the profiling path.
bass_utils.upload_artifacts = lambda tmpdir: f"file://{tmpdir}"

B = 2
S = 4096
D = 512
H = 8
DK = 64
HPC = 2          # heads per core
NCH = 4          # D / 128 contraction chunks
NKT = S // 128   # 32 k-tiles
SQB = 512        # phase-B q strip width
NSTRIP = S // SQB
PBLK = 512       # phase-A projection q block
KTE = 2          # k-tiles per PSUM tile / exp instruction

F32 = mybir.dt.float32
F32R = mybir.dt.float32r
F16 = mybir.dt.float16
PENALTY = -30000.0  # fits fp16; exp(S + PENALTY) == 0 in fp32

_NC_CACHE = None


def _f32(ap):
    return ap.bitcast(F32)


def _bcast_mid(ap, n):
    """Insert a step-0 middle free dim of extent n into a 2D AP."""
    return bass.AP(tensor=ap.tensor, offset=ap.offset, ap=[ap.ap[0], [0, n], ap.ap[1]])


def build_nc():
    nc = bacc.Bacc("TRN2", target_bir_lowering=False, debug=False, num_devices=8)

    qT = nc.dram_tensor("qT", [128, NCH, S], F16, kind="ExternalInput")
    kT = nc.dram_tensor("kT", [128, NCH, S], F16, kind="ExternalInput")
    vT = nc.dram_tensor("vT", [128, NCH, S], F16, kind="ExternalInput")
    wq = nc.dram_tensor("wq", [128, NCH, HPC, DK], F16, kind="ExternalInput")
    wk = nc.dram_tensor("wk", [128, NCH, HPC, DK], F16, kind="ExternalInput")
    wv = nc.dram_tensor("wv", [128, NCH, HPC, DK], F16, kind="ExternalInput")
    wo = nc.dram_tensor("wo", [DK, HPC, NCH, 128], F16, kind="ExternalInput")
    # aux[0] = ones (Q^T row 64), aux[1] = mask penalty (K^T row 64)
    aux = nc.dram_tensor("aux", [2, 1, HPC, S], F16, kind="ExternalInput")

    attn_t = nc.dram_tensor("attn_t", [HPC, NKT, 128, S], F16, kind="ExternalOutput")
    out_t = nc.dram_tensor("out_t", [NCH, 128, S], F32, kind="ExternalOutput")

    from contextlib import ExitStack
    with tile.TileContext(nc) as tc, ExitStack() as stack:
        # ---- persistent SBUF ----
        persist = stack.enter_context(tc.tile_pool(name="persist", bufs=1))
        QTp = persist.tile([128, HPC, S], F16)  # rows 0-63 Q^T, row 64 ones, rest 0
        KTp = persist.tile([128, HPC, S], F16)  # rows 0-63 K^T, row 64 penalty
        Vsb = persist.tile([128, NKT, HPC, 65], F16)  # cols 0-63 V, col 64 ones
        ctx_sb = persist.tile([DK, HPC, S], F16)
        wq_sb = persist.tile([128, NCH, HPC, DK], F16)
        wk_sb = persist.tile([128, NCH, HPC, DK], F16)
        wv_sb = persist.tile([128, NCH, HPC, DK], F16)
        wo_sb = persist.tile([DK, HPC, NCH, 128], F16)

        nc.scalar.dma_start(out=wq_sb[:], in_=wq[:])
        nc.scalar.dma_start(out=wk_sb[:], in_=wk[:])
        nc.scalar.dma_start(out=wv_sb[:], in_=wv[:])
        nc.scalar.dma_start(out=wo_sb[:], in_=wo[:])
        nc.vector.memset(Vsb[:, :, :, 64:65], 1.0)
        # zero the padding rows so the 128-deep contraction is exact (and the
        # full-depth weight load path can engage); row 64 is then overwritten
        # by the aux DMAs below
        nc.vector.memset(QTp[64:128, :, :], 0.0)
        nc.vector.memset(KTp[64:128, :, :], 0.0)
        nc.scalar.dma_start(out=QTp[64:65, :, :], in_=aux[0, :, :, :])
        nc.scalar.dma_start(out=KTp[64:65, :, :], in_=aux[1, :, :, :])

        # ---- phase A: projections (fp16 matmuls) ----
        with tc.tile_pool(name="stage", bufs=6) as stage, \
             tc.tile_pool(name="pps", bufs=3, space="PSUM") as pps:
            for (src, w_sb, dstp) in ((qT, wq_sb, QTp), (kT, wk_sb, KTp)):
                chunks = []
                for c in range(NCH):
                    t = stage.tile([128, S], F16, tag="stage")
                    nc.scalar.dma_start(out=t[:], in_=src[:, c, :])
                    chunks.append(t)
                for h in range(HPC):
                    for pb in range(S // PBLK):
                        ps = pps.tile([DK, PBLK], F32, tag="proj")
                        for c in range(NCH):
                            nc.tensor.matmul(
                                ps[:],
                                w_sb[:, c, h, :],
                                chunks[c][:, pb * PBLK:(pb + 1) * PBLK],
                                start=(c == 0), stop=(c == NCH - 1),
                            )
                        nc.vector.tensor_copy(
                            out=dstp[0:DK, h, pb * PBLK:(pb + 1) * PBLK], in_=ps[:]
                        )
            # V = value @ Wv^T, built [k, dv] with k on partitions
            chunks = []
            for c in range(NCH):
                t = stage.tile([128, S], F16, tag="stage")
                nc.scalar.dma_start(out=t[:], in_=vT[:, c, :])
                chunks.append(t)
            for kt in range(NKT):
                ps = pps.tile([128, HPC * DK], F32, tag="vproj")
                for c in range(NCH):
                    nc.tensor.matmul(
                        ps[:],
                        chunks[c][:, kt * 128:(kt + 1) * 128],
                        wv_sb[:, c, :, :],
                        start=(c == 0), stop=(c == NCH - 1),
                    )
                for h in range(HPC):
                    nc.vector.tensor_copy(
                        out=Vsb[:, kt, h, 0:DK], in_=ps[:, h * DK:(h + 1) * DK]
                    )

        # ---- phase B: scores, softmax, context, interleaved out-proj ----
        with tc.tile_pool(name="epool", bufs=2) as epool, \
             tc.tile_pool(name="small", bufs=3) as small, \
             tc.tile_pool(name="invp", bufs=2) as invp, \
             tc.tile_pool(name="oblk", bufs=2) as oblk, \
             tc.tile_pool(name="sps", bufs=2, space="PSUM") as sps, \
             tc.tile_pool(name="cps", bufs=2, space="PSUM") as cps, \
             tc.tile_pool(name="ops", bufs=2, space="PSUM") as ops:

            def emit_out_proj(q0):
                """Output projection for q-block q0 (both heads' ctx ready)."""
                ob = oblk.tile([128, NCH, PBLK], F32, tag="ob")
                for c in range(NCH):
                    ps = ops.tile([128, PBLK], F32, tag="o")
                    for h in range(HPC):
                        nc.tensor.matmul(
                            ps[:],
                            wo_sb[:, h, c, :],
                            ctx_sb[:, h, q0:q0 + PBLK],
                            start=(h == 0), stop=(h == HPC - 1),
                        )
                    nc.scalar.copy(out=ob[:, c, :], in_=ps[:])
                nc.sync.dma_start(
                    out=out_t[:].rearrange("c p q -> p c q")[:, :, q0:q0 + PBLK],
                    in_=ob[:],
                )

            for sb in range(NSTRIP):
                q0 = sb * SQB
                for h in range(HPC):
                    E = epool.tile([128, NKT, SQB], F16, tag="E")
                    cx = cps.tile([65, SQB], F32, tag="ctx")

                    def emit_ctx(kte):
                        for j in range(KTE):
                            kt = kte * KTE + j
                            nc.tensor.matmul(
                                cx[:],
                                Vsb[:, kt, h, 0:65],
                                E[:, kt, :],
                                start=(kt == 0), stop=(kt == NKT - 1),
                            )

                    # software pipeline: ctx matmuls run one kte group behind
                    # the score matmuls so TensorE never waits on an exp.
                    for kte in range(NKT // KTE):
                        sp = sps.tile([128, KTE, SQB], F32, tag="s")
                        for j in range(KTE):
                            kt = kte * KTE + j
                            nc.tensor.matmul(
                                sp[:, j, :],
                                KTp[:, h, kt * 128:(kt + 1) * 128],
                                QTp[:, h, q0:q0 + SQB],
                                start=True, stop=True,
                            )
                        nc.scalar.activation(
                            out=E[:, kte * KTE:(kte + 1) * KTE, :],
                            in_=sp[:],
                            func=mybir.ActivationFunctionType.Exp,
                        )
                        if kte > 0:
                            emit_ctx(kte - 1)
                    emit_ctx(NKT // KTE - 1)
                    # strip epilogue: no TensorE work in here
                    inv = small.tile([1, SQB], F16, tag="inv")
                    with nc.allow_low_precision(reason="fp16 softmax normalize"):
                        nc.vector.reciprocal(out=inv[:], in_=cx[64:65, :])
                    ibs = invp.tile([128, SQB], F16, tag="ibs")
                    nc.gpsimd.partition_broadcast(ibs[:], inv[:])
                    with nc.allow_low_precision(reason="fp16 P/ctx tiles"):
                        nc.vector.tensor_mul(
                            ctx_sb[:, h, q0:q0 + SQB], cx[0:DK, :], ibs[0:DK, :]
                        )
                        nc.vector.tensor_mul(
                            E[:], E[:], _bcast_mid(ibs[:], NKT)
                        )
                    nc.sync.dma_start(
                        out=attn_t[h].rearrange("kt p q -> p kt q")[:, :, q0:q0 + SQB],
                        in_=E[:],
                    )
                if sb > 0:
                    emit_out_proj((sb - 1) * SQB)
            emit_out_proj((NSTRIP - 1) * SQB)

    nc.compile()
    return nc


def _get_nc():
    global _NC_CACHE
    if _NC_CACHE is None:
        _NC_CACHE = build_nc()
    return _NC_CACHE


def _prep_inputs(query, key, value, mask, W_q, W_k, W_v, W_o):
    """Build the 8 per-core input dicts."""
    SCALE = np.float32(1.0 / np.sqrt(DK))
    xt = {}
    for b in range(B):
        for name, arr in (("qT", query), ("kT", key), ("vT", value)):
            t = np.ascontiguousarray(
                arr[b].T.reshape(NCH, 128, S).transpose(1, 0, 2).astype(np.float16)
            )
            xt[(name, b)] = t
    pen = [
        np.where(mask[b, 0, 0] == 0, np.float32(PENALTY), np.float32(0.0)).astype(
            np.float16
        )
        for b in range(B)
    ]
    in_maps = []
    for core in range(8):
        b, hp = divmod(core, 4)
        h0 = hp * HPC
        sl = slice(h0 * DK, (h0 + HPC) * DK)

        def wslice(W, scale=None):
            ws = W[sl]  # [128, 512] rows = head outputs
            if scale is not None:
                ws = ws * scale
            # [p, c, h, j] = ws[h*64+j, c*128+p]
            return np.ascontiguousarray(
                ws.reshape(HPC, DK, NCH, 128).transpose(3, 2, 0, 1)
            ).astype(np.float16)

        wo_arr = np.ascontiguousarray(
            W_o[:, sl].T.reshape(HPC, DK, NCH, 128).transpose(1, 0, 2, 3)
        ).astype(np.float16)
        auxa = np.empty((2, 1, HPC, S), np.float16)
        auxa[0] = np.float16(1.0)
        auxa[1, 0, :, :] = pen[b][None, :]
        in_maps.append({
            "qT": xt[("qT", b)],
            "kT": xt[("kT", b)],
            "vT": xt[("vT", b)],
            "wq": wslice(W_q, SCALE),
            "wk": wslice(W_k),
            "wv": wslice(W_v),
            "wo": wo_arr,
            "aux": auxa,
        })
    return in_maps


def kernel(query, key, value, mask, W_q, W_k, W_v, W_o, b_o, _trace=False,
           _trace_kwargs=None):
    query = np.asarray(query, np.float32)
    key = np.asarray(key, np.float32)
    value = np.asarray(value, np.float32)
    mask = np.asarray(mask)
    W_q = np.asarray(W_q, np.float32)
    W_k = np.asarray(W_k, np.float32)
    W_v = np.asarray(W_v, np.float32)
    W_o = np.asarray(W_o, np.float32)
    b_o = np.asarray(b_o, np.float32)

    nc = _get_nc()
    in_maps = _prep_inputs(query, key, value, mask, W_q, W_k, W_v, W_o)
    kw = dict(_trace_kwargs or {})
    res = run_bass_kernel_spmd(nc, in_maps, core_ids=list(range(8)),
                               trace=_trace, **kw)
    kernel.last_result = res

    attn = np.empty((B, H, S, S), np.float32)
    out = np.zeros((B, S, D), np.float32)
    for core in range(8):
        b, hp = divmod(core, 4)
        r = res.results[core]
        for h in range(HPC):
            attn[b, hp * HPC + h] = r["attn_t"][h].reshape(S, S).T
        out[b] += r["out_t"].reshape(D, S).T
    out += b_o
    return out, attn


# revision 19
# speedup vs baseline: 1.8893x; 1.0656x over previous
"""Trainium2 Bass kernel for nn_MultiHeadAttention (B=2, S=4096, D=512, H=8).

Sharding: 8 cores = (2 batches) x (4 head-pairs). Each core computes two
heads' attention for one batch plus its partial output projection.

On-chip orientation is "k-major": S^T[k,q] = K @ Q^T is computed with k on
partitions, so softmax row-sums reduce over the partition axis — fused into
the context matmul via a ones column in V — and the context matmul needs no
transposes at all. The padding mask is folded into the score matmul as an
extra contraction row (lhsT row 64 = penalty, rhs row 64 = 1.0). exp runs on
ScalarE directly out of PSUM; the 1/rowsum vector is broadcast across
partitions by GpSimd so TensorE never stalls on strip epilogues. The output
projection is interleaved into the strip loop one strip behind. attn is
written to HBM fp16 in [k,q] layout and transposed + upcast on the host
during unsharding.

All matmuls are fp16 with fp32 PSUM accumulation.
"""

import os
import sys

sys.path.insert(0, "/opt/trn_rl_repo")

import numpy as np

import concourse.bass as bass
import concourse.mybir as mybir
import concourse.tile as tile
from concourse import bacc
from concourse.bass_utils import run_bass_kernel_spmd
import concourse.bass_utils as bass_utils

# Avoid S3 artifact uploads from the profiling path.
bass_utils.upload_artifacts = lambda tmpdir: f"file://{tmpdir}"


B = 2
S = 4096
D = 512
H = 8
DK = 64
HPC = 2          # heads per core
NCH = 4          # D / 128 contraction chunks
NKT = S // 128   # 32 k-tiles
SQB = 512        # phase-B q strip width
NSTRIP = S // SQB
PBLK = 512       # phase-A projection q block
KTE = 2          # k-tiles per PSUM tile / exp instruction

F32 = mybir.dt.float32
F32R = mybir.dt.float32r
F16 = mybir.dt.float16
PENALTY = -30000.0  # fits fp16; exp(S + PENALTY) == 0 in fp32

_NC_CACHE = None


def _f32(ap):
    return ap.bitcast(F32)


def _bcast_mid(ap, n):
    """Insert a step-0 middle free dim of extent n into a 2D AP."""
    return bass.AP(tensor=ap.tensor, offset=ap.offset, ap=[ap.ap[0], [0, n], ap.ap[1]])


def build_nc():
    nc = bacc.Bacc("TRN2", target_bir_lowering=False, debug=False, num_devices=8)

    qT = nc.dram_tensor("qT", [128, NCH, S], F16, kind="ExternalInput")
    kT = nc.dram_tensor("kT", [128, NCH, S], F16, kind="ExternalInput")
    vT = nc.dram_tensor("vT", [128, NCH, S], F16, kind="ExternalInput")
    wq = nc.dram_tensor("wq", [128, NCH, HPC, DK], F16, kind="ExternalInput")
    wk = nc.dram_tensor("wk", [128, NCH, HPC, DK], F16, kind="ExternalInput")
    wv = nc.dram_tensor("wv", [128, NCH, HPC, DK], F16, kind="ExternalInput")
    wo = nc.dram_tensor("wo", [DK, HPC, NCH, 128], F16, kind="ExternalInput")
    # aux[0] = ones (Q^T row 64), aux[1] = mask penalty (K^T row 64)
    aux = nc.dram_tensor("aux", [2, 1, HPC, S], F16, kind="ExternalInput")

    attn_t = nc.dram_tensor("attn_t", [HPC, NKT, 128, S], F16, kind="ExternalOutput")
    out_t = nc.dram_tensor("out_t", [NCH, 128, S], F32, kind="ExternalOutput")

    from contextlib import ExitStack
    with tile.TileContext(nc) as tc, ExitStack() as stack:
        # ---- persistent SBUF ----
        persist = stack.enter_context(tc.tile_pool(name="persist", bufs=1))
        QTp = persist.tile([128, HPC, S], F16)  # rows 0-63 Q^T, row 64 ones, rest 0
        KTp = persist.tile([128, HPC, S], F16)  # rows 0-63 K^T, row 64 penalty
        Vsb = persist.tile([128, NKT, HPC, 65], F16)  # cols 0-63 V, col 64 ones
        ctx_sb = persist.tile([DK, HPC, S], F16)
        wq_sb = persist.tile([128, NCH, HPC, DK], F16)
        wk_sb = persist.tile([128, NCH, HPC, DK], F16)
        wv_sb = persist.tile([128, NCH, HPC, DK], F16)
        wo_sb = persist.tile([DK, HPC, NCH, 128], F16)

        nc.scalar.dma_start(out=wq_sb[:], in_=wq[:])
        nc.scalar.dma_start(out=wk_sb[:], in_=wk[:])
        nc.scalar.dma_start(out=wv_sb[:], in_=wv[:])
        nc.scalar.dma_start(out=wo_sb[:], in_=wo[:])
        nc.vector.memset(Vsb[:, :, :, 64:65], 1.0)
        # zero the padding rows so the 128-deep contraction is exact (and the
        # full-depth weight load path can engage); row 64 is then overwritten
        # by the aux DMAs below
        nc.vector.memset(QTp[64:128, :, :], 0.0)
        nc.vector.memset(KTp[64:128, :, :], 0.0)
        nc.scalar.dma_start(out=QTp[64:65, :, :], in_=aux[0, :, :, :])
        nc.scalar.dma_start(out=KTp[64:65, :, :], in_=aux[1, :, :, :])

        # ---- phase A: projections (fp16 matmuls) ----
        with tc.tile_pool(name="stage", bufs=6) as stage, \
             tc.tile_pool(name="pps", bufs=3, space="PSUM") as pps:
            for (src, w_sb, dstp) in ((qT, wq_sb, QTp), (kT, wk_sb, KTp)):
                chunks = []
                for c in range(NCH):
                    t = stage.tile([128, S], F16, tag="stage")
                    nc.scalar.dma_start(out=t[:], in_=src[:, c, :])
                    chunks.append(t)
                for h in range(HPC):
                    for pb in range(S // PBLK):
                        ps = pps.tile([DK, PBLK], F32, tag="proj")
                        for c in range(NCH):
                            nc.tensor.matmul(
                                ps[:],
                                w_sb[:, c, h, :],
                                chunks[c][:, pb * PBLK:(pb + 1) * PBLK],
                                start=(c == 0), stop=(c == NCH - 1),
                            )
                        nc.vector.tensor_copy(
                            out=dstp[0:DK, h, pb * PBLK:(pb + 1) * PBLK], in_=ps[:]
                        )
            # V = value @ Wv^T, built [k, dv] with k on partitions
            chunks = []
            for c in range(NCH):
                t = stage.tile([128, S], F16, tag="stage")
                nc.scalar.dma_start(out=t[:], in_=vT[:, c, :])
                chunks.append(t)
            for kt in range(NKT):
                ps = pps.tile([128, HPC * DK], F32, tag="vproj")
                for c in range(NCH):
                    nc.tensor.matmul(
                        ps[:],
                        chunks[c][:, kt * 128:(kt + 1) * 128],
                        wv_sb[:, c, :, :],
                        start=(c == 0), stop=(c == NCH - 1),
                    )
                for h in range(HPC):
                    nc.vector.tensor_copy(
                        out=Vsb[:, kt, h, 0:DK], in_=ps[:, h * DK:(h + 1) * DK]
                    )

        # ---- phase B: scores, softmax, context, interleaved out-proj ----
        with tc.tile_pool(name="epool", bufs=2) as epool, \
             tc.tile_pool(name="small", bufs=3) as small, \
             tc.tile_pool(name="invp", bufs=2) as invp, \
             tc.tile_pool(name="oblk", bufs=2) as oblk, \
             tc.tile_pool(name="sps", bufs=2, space="PSUM") as sps, \
             tc.tile_pool(name="cps", bufs=2, space="PSUM") as cps, \
             tc.tile_pool(name="ops", bufs=2, space="PSUM") as ops:

            def emit_out_proj(q0):
                """Output projection for q-block q0 (both heads' ctx ready)."""
                ob = oblk.tile([128, NCH, PBLK], F32, tag="ob")
                for c in range(NCH):
                    ps = ops.tile([128, PBLK], F32, tag="o")
                    for h in range(HPC):
                        nc.tensor.matmul(
                            ps[:],
                            wo_sb[:, h, c, :],
                            ctx_sb[:, h, q0:q0 + PBLK],
                            start=(h == 0), stop=(h == HPC - 1),
                        )
                    nc.scalar.copy(out=ob[:, c, :], in_=ps[:])
                nc.sync.dma_start(
                    out=out_t[:].rearrange("c p q -> p c q")[:, :, q0:q0 + PBLK],
                    in_=ob[:],
                )

            for sb in range(NSTRIP):
                q0 = sb * SQB
                for h in range(HPC):
                    E = epool.tile([128, NKT, SQB], F16, tag="E")
                    cx = cps.tile([65, SQB], F32, tag="ctx")

                    def emit_ctx(kte):
                        for j in range(KTE):
                            kt = kte * KTE + j
                            nc.tensor.matmul(
                                cx[:],
                                Vsb[:, kt, h, 0:65],
                                E[:, kt, :],
                                start=(kt == 0), stop=(kt == NKT - 1),
                            )

                    # software pipeline: ctx matmuls run two kte groups behind
                    # the score matmuls so TensorE never waits on an exp.
                    DEPTH = 2
                    for kte in range(NKT // KTE):
                        sp = sps.tile([128, KTE, SQB], F32, tag="s")
                        for j in range(KTE):
                            kt = kte * KTE + j
                            nc.tensor.matmul(
                                sp[:, j, :],
                                KTp[:, h, kt * 128:(kt + 1) * 128],
                                QTp[:, h, q0:q0 + SQB],
                                start=True, stop=True,
                            )
                        nc.scalar.activation(
                            out=E[:, kte * KTE:(kte + 1) * KTE, :],
                            in_=sp[:],
                            func=mybir.ActivationFunctionType.Exp,
                        )
                        if kte >= DEPTH:
                            emit_ctx(kte - DEPTH)
                    for kte in range(NKT // KTE - DEPTH, NKT // KTE):
                        emit_ctx(kte)
                    # strip epilogue: no TensorE work in here
                    inv = small.tile([1, SQB], F16, tag="inv")
                    with nc.allow_low_precision(reason="fp16 softmax normalize"):
                        nc.vector.reciprocal(out=inv[:], in_=cx[64:65, :])
                    ibs = invp.tile([128, SQB], F16, tag="ibs")
                    nc.gpsimd.partition_broadcast(ibs[:], inv[:])
                    with nc.allow_low_precision(reason="fp16 P/ctx tiles"):
                        nc.vector.tensor_mul(
                            ctx_sb[:, h, q0:q0 + SQB], cx[0:DK, :], ibs[0:DK, :]
                        )
                        nc.vector.tensor_mul(
                            E[:], E[:], _bcast_mid(ibs[:], NKT)
                        )
                    nc.sync.dma_start(
                        out=attn_t[h].rearrange("kt p q -> p kt q")[:, :, q0:q0 + SQB],
                        in_=E[:],
                    )
                if sb > 0:
                    emit_out_proj((sb - 1) * SQB)
            emit_out_proj((NSTRIP - 1) * SQB)

    nc.compile()
    return nc


def _get_nc():
    global _NC_CACHE
    if _NC_CACHE is None:
        _NC_CACHE = build_nc()
    return _NC_CACHE


def _prep_inputs(query, key, value, mask, W_q, W_k, W_v, W_o):
    """Build the 8 per-core input dicts."""
    SCALE = np.float32(1.0 / np.sqrt(DK))
    xt = {}
    for b in range(B):
        for name, arr in (("qT", query), ("kT", key), ("vT", value)):
            t = np.ascontiguousarray(
                arr[b].T.reshape(NCH, 128, S).transpose(1, 0, 2).astype(np.float16)
            )
            xt[(name, b)] = t
    pen = [
        np.where(mask[b, 0, 0] == 0, np.float32(PENALTY), np.float32(0.0)).astype(
            np.float16
        )
        for b in range(B)
    ]
    in_maps = []
    for core in range(8):
        b, hp = divmod(core, 4)
        h0 = hp * HPC
        sl = slice(h0 * DK, (h0 + HPC) * DK)

        def wslice(W, scale=None):
            ws = W[sl]  # [128, 512] rows = head outputs
            if scale is not None:
                ws = ws * scale
            # [p, c, h, j] = ws[h*64+j, c*128+p]
            return np.ascontiguousarray(
                ws.reshape(HPC, DK, NCH, 128).transpose(3, 2, 0, 1)
            ).astype(np.float16)

        wo_arr = np.ascontiguousarray(
            W_o[:, sl].T.reshape(HPC, DK, NCH, 128).transpose(1, 0, 2, 3)
        ).astype(np.float16)
        auxa = np.empty((2, 1, HPC, S), np.float16)
        auxa[0] = np.float16(1.0)
        auxa[1, 0, :, :] = pen[b][None, :]
        in_maps.append({
            "qT": xt[("qT", b)],
            "kT": xt[("kT", b)],
            "vT": xt[("vT", b)],
            "wq": wslice(W_q, SCALE),
            "wk": wslice(W_k),
            "wv": wslice(W_v),
            "wo": wo_arr,
            "aux": auxa,
        })
    return in_maps


def kernel(query, key, value, mask, W_q, W_k, W_v, W_o, b_o, _trace=False,
           _trace_kwargs=None):
    query = np.asarray(query, np.float32)
    key = np.asarray(key, np.float32)
    value = np.asarray(value, np.float32)
    mask = np.asarray(mask)
    W_q = np.asarray(W_q, np.float32)
    W_k = np.asarray(W_k, np.float32)
    W_v = np.asarray(W_v, np.float32)
    W_o = np.asarray(W_o, np.float32)
    b_o = np.asarray(b_o, np.float32)

    nc = _get_nc()
    in_maps = _prep_inputs(query, key, value, mask, W_q, W_k, W_v, W_o)
    kw = dict(_trace_kwargs or {})
    res = run_bass_kernel_spmd(nc, in_maps, core_ids=list(range(8)),
                               trace=_trace, **kw)
    kernel.last_result = res

    attn = np.empty((B, H, S, S), np.float32)
    out = np.zeros((B, S, D), np.float32)
    for core in range(8):
        b, hp = divmod(core, 4)
        r = res.results[core]
        for h in range(HPC):
            attn[b, hp * HPC + h] = r["attn_t"][h].reshape(S, S).T
        out[b] += r["out_t"].reshape(D, S).T
    out += b_o
    return out, attn


# revision 20
# speedup vs baseline: 1.9102x; 1.0111x over previous
"""Trainium2 Bass kernel for nn_MultiHeadAttention (B=2, S=4096, D=512, H=8).

Sharding: 8 cores = (2 batches) x (4 head-pairs). Each core computes two
heads' attention for one batch plus its partial output projection.

On-chip orientation is "k-major": S^T[k,q] = K @ Q^T is computed with k on
partitions, so softmax row-sums reduce over the partition axis — fused into
the context matmul via a ones column in V — and the context matmul needs no
transposes at all. The padding mask is folded into the score matmul as an
extra contraction row (lhsT row 64 = penalty, rhs row 64 = 1.0). exp runs on
ScalarE directly out of PSUM; the 1/rowsum vector is broadcast across
partitions by GpSimd so TensorE never stalls on strip epilogues. The output
projection is interleaved into the strip loop one strip behind. attn is
written to HBM fp16 in [k,q] layout and transposed + upcast on the host
during unsharding.

All matmuls are fp16 with fp32 PSUM accumulation.
"""

import os
import sys

sys.path.insert(0, "/opt/trn_rl_repo")

import numpy as np

import concourse.bass as bass
import concourse.mybir as mybir
import concourse.tile as tile
from concourse import bacc
from concourse.bass_utils import run_bass_kernel_spmd
import concourse.bass_utils as bass_utils

# Avoid S3 artifact uploads from the profiling path.
bass_utils.upload_artifacts = lambda tmpdir: f"file://{tmpdir}"


B = 2
S = 4096
D = 512
H = 8
DK = 64
HPC = 2          # heads per core
NCH = 4          # D / 128 contraction chunks
NKT = S // 128   # 32 k-tiles
SQB = 512        # phase-B q strip width
NSTRIP = S // SQB
PBLK = 512       # phase-A projection q block
KTE = 2          # k-tiles per PSUM tile / exp instruction

F32 = mybir.dt.float32
F32R = mybir.dt.float32r
F16 = mybir.dt.float16
PENALTY = -30000.0  # fits fp16; exp(S + PENALTY) == 0 in fp32

_NC_CACHE = None


def _f32(ap):
    return ap.bitcast(F32)


def _bcast_mid(ap, n):
    """Insert a step-0 middle free dim of extent n into a 2D AP."""
    return bass.AP(tensor=ap.tensor, offset=ap.offset, ap=[ap.ap[0], [0, n], ap.ap[1]])


def build_nc():
    nc = bacc.Bacc("TRN2", target_bir_lowering=False, debug=False, num_devices=8)

    qT = nc.dram_tensor("qT", [128, NCH, S], F16, kind="ExternalInput")
    kT = nc.dram_tensor("kT", [128, NCH, S], F16, kind="ExternalInput")
    vT = nc.dram_tensor("vT", [128, NCH, S], F16, kind="ExternalInput")
    wq = nc.dram_tensor("wq", [128, NCH, HPC, DK], F16, kind="ExternalInput")
    wk = nc.dram_tensor("wk", [128, NCH, HPC, DK], F16, kind="ExternalInput")
    wv = nc.dram_tensor("wv", [128, NCH, HPC, DK], F16, kind="ExternalInput")
    wo = nc.dram_tensor("wo", [DK, HPC, NCH, 128], F16, kind="ExternalInput")
    # aux[0] = ones (Q^T row 64), aux[1] = mask penalty (K^T row 64)
    aux = nc.dram_tensor("aux", [2, 1, HPC, S], F16, kind="ExternalInput")

    attn_t = nc.dram_tensor("attn_t", [HPC, NKT, 128, S], F16, kind="ExternalOutput")
    out_t = nc.dram_tensor("out_t", [NCH, 128, S], F32, kind="ExternalOutput")

    from contextlib import ExitStack
    with tile.TileContext(nc) as tc, ExitStack() as stack:
        # ---- persistent SBUF ----
        persist = stack.enter_context(tc.tile_pool(name="persist", bufs=1))
        QTp = persist.tile([128, HPC, S], F16)  # rows 0-63 Q^T, row 64 ones, rest 0
        KTp = persist.tile([128, HPC, S], F16)  # rows 0-63 K^T, row 64 penalty
        Vsb = persist.tile([128, NKT, HPC, 65], F16)  # cols 0-63 V, col 64 ones
        ctx_sb = persist.tile([DK, HPC, S], F16)
        wq_sb = persist.tile([128, NCH, HPC, DK], F16)
        wk_sb = persist.tile([128, NCH, HPC, DK], F16)
        wv_sb = persist.tile([128, NCH, HPC, DK], F16)
        wo_sb = persist.tile([DK, HPC, NCH, 128], F16)

        nc.scalar.dma_start(out=wq_sb[:], in_=wq[:])
        nc.scalar.dma_start(out=wk_sb[:], in_=wk[:])
        nc.scalar.dma_start(out=wv_sb[:], in_=wv[:])
        nc.scalar.dma_start(out=wo_sb[:], in_=wo[:])
        nc.vector.memset(Vsb[:, :, :, 64:65], 1.0)
        # zero the padding rows so the 128-deep contraction is exact (and the
        # full-depth weight load path can engage); row 64 is then overwritten
        # by the aux DMAs below
        nc.vector.memset(QTp[64:128, :, :], 0.0)
        nc.vector.memset(KTp[64:128, :, :], 0.0)
        nc.scalar.dma_start(out=QTp[64:65, :, :], in_=aux[0, :, :, :])
        nc.scalar.dma_start(out=KTp[64:65, :, :], in_=aux[1, :, :, :])

        # ---- phase A: projections (fp16 matmuls) ----
        with tc.tile_pool(name="stage", bufs=6) as stage, \
             tc.tile_pool(name="pps", bufs=3, space="PSUM") as pps:
            for (src, w_sb, dstp) in ((qT, wq_sb, QTp), (kT, wk_sb, KTp)):
                chunks = []
                for c in range(NCH):
                    t = stage.tile([128, S], F16, tag="stage")
                    nc.scalar.dma_start(out=t[:], in_=src[:, c, :])
                    chunks.append(t)
                for pb in range(S // PBLK):
                    ps = pps.tile([HPC * DK, PBLK], F32, tag="proj")
                    for c in range(NCH):
                        nc.tensor.matmul(
                            ps[:],
                            w_sb[:, c, :, :],
                            chunks[c][:, pb * PBLK:(pb + 1) * PBLK],
                            start=(c == 0), stop=(c == NCH - 1),
                        )
                    for h in range(HPC):
                        nc.vector.tensor_copy(
                            out=dstp[0:DK, h, pb * PBLK:(pb + 1) * PBLK],
                            in_=ps[h * DK:(h + 1) * DK, :],
                        )
            # V = value @ Wv^T, built [k, dv] with k on partitions
            chunks = []
            for c in range(NCH):
                t = stage.tile([128, S], F16, tag="stage")
                nc.scalar.dma_start(out=t[:], in_=vT[:, c, :])
                chunks.append(t)
            for kt in range(NKT):
                ps = pps.tile([128, HPC * DK], F32, tag="vproj")
                for c in range(NCH):
                    nc.tensor.matmul(
                        ps[:],
                        chunks[c][:, kt * 128:(kt + 1) * 128],
                        wv_sb[:, c, :, :],
                        start=(c == 0), stop=(c == NCH - 1),
                    )
                for h in range(HPC):
                    nc.vector.tensor_copy(
                        out=Vsb[:, kt, h, 0:DK], in_=ps[:, h * DK:(h + 1) * DK]
                    )

        # ---- phase B: scores, softmax, context, interleaved out-proj ----
        with tc.tile_pool(name="epool", bufs=2) as epool, \
             tc.tile_pool(name="small", bufs=3) as small, \
             tc.tile_pool(name="invp", bufs=2) as invp, \
             tc.tile_pool(name="oblk", bufs=2) as oblk, \
             tc.tile_pool(name="sps", bufs=2, space="PSUM") as sps, \
             tc.tile_pool(name="cps", bufs=2, space="PSUM") as cps, \
             tc.tile_pool(name="ops", bufs=2, space="PSUM") as ops:

            def emit_out_proj(q0):
                """Output projection for q-block q0 (both heads' ctx ready)."""
                ob = oblk.tile([128, NCH, PBLK], F32, tag="ob")
                for c in range(NCH):
                    ps = ops.tile([128, PBLK], F32, tag="o")
                    for h in range(HPC):
                        nc.tensor.matmul(
                            ps[:],
                            wo_sb[:, h, c, :],
                            ctx_sb[:, h, q0:q0 + PBLK],
                            start=(h == 0), stop=(h == HPC - 1),
                        )
                    nc.scalar.copy(out=ob[:, c, :], in_=ps[:])
                nc.sync.dma_start(
                    out=out_t[:].rearrange("c p q -> p c q")[:, :, q0:q0 + PBLK],
                    in_=ob[:],
                )

            for sb in range(NSTRIP):
                q0 = sb * SQB
                for h in range(HPC):
                    E = epool.tile([128, NKT, SQB], F16, tag="E")
                    cx = cps.tile([65, SQB], F32, tag="ctx")

                    def emit_ctx(kte):
                        for j in range(KTE):
                            kt = kte * KTE + j
                            nc.tensor.matmul(
                                cx[:],
                                Vsb[:, kt, h, 0:65],
                                E[:, kt, :],
                                start=(kt == 0), stop=(kt == NKT - 1),
                            )

                    # software pipeline: ctx matmuls run two kte groups behind
                    # the score matmuls so TensorE never waits on an exp.
                    DEPTH = 2
                    for kte in range(NKT // KTE):
                        sp = sps.tile([128, KTE, SQB], F32, tag="s")
                        for j in range(KTE):
                            kt = kte * KTE + j
                            nc.tensor.matmul(
                                sp[:, j, :],
                                KTp[:, h, kt * 128:(kt + 1) * 128],
                                QTp[:, h, q0:q0 + SQB],
                                start=True, stop=True,
                            )
                        nc.scalar.activation(
                            out=E[:, kte * KTE:(kte + 1) * KTE, :],
                            in_=sp[:],
                            func=mybir.ActivationFunctionType.Exp,
                        )
                        if kte >= DEPTH:
                            emit_ctx(kte - DEPTH)
                    for kte in range(NKT // KTE - DEPTH, NKT // KTE):
                        emit_ctx(kte)
                    # strip epilogue: no TensorE work in here
                    inv = small.tile([1, SQB], F16, tag="inv")
                    with nc.allow_low_precision(reason="fp16 softmax normalize"):
                        nc.vector.reciprocal(out=inv[:], in_=cx[64:65, :])
                    ibs = invp.tile([128, SQB], F16, tag="ibs")
                    nc.gpsimd.partition_broadcast(ibs[:], inv[:])
                    with nc.allow_low_precision(reason="fp16 P/ctx tiles"):
                        nc.vector.tensor_mul(
                            ctx_sb[:, h, q0:q0 + SQB], cx[0:DK, :], ibs[0:DK, :]
                        )
                        nc.vector.tensor_mul(
                            E[:], E[:], _bcast_mid(ibs[:], NKT)
                        )
                    nc.sync.dma_start(
                        out=attn_t[h].rearrange("kt p q -> p kt q")[:, :, q0:q0 + SQB],
                        in_=E[:],
                    )
                if sb > 0:
                    emit_out_proj((sb - 1) * SQB)
            emit_out_proj((NSTRIP - 1) * SQB)

    nc.compile()
    return nc


def _get_nc():
    global _NC_CACHE
    if _NC_CACHE is None:
        _NC_CACHE = build_nc()
    return _NC_CACHE


def _prep_inputs(query, key, value, mask, W_q, W_k, W_v, W_o):
    """Build the 8 per-core input dicts."""
    SCALE = np.float32(1.0 / np.sqrt(DK))
    xt = {}
    for b in range(B):
        for name, arr in (("qT", query), ("kT", key), ("vT", value)):
            t = np.ascontiguousarray(
                arr[b].T.reshape(NCH, 128, S).transpose(1, 0, 2).astype(np.float16)
            )
            xt[(name, b)] = t
    pen = [
        np.where(mask[b, 0, 0] == 0, np.float32(PENALTY), np.float32(0.0)).astype(
            np.float16
        )
        for b in range(B)
    ]
    in_maps = []
    for core in range(8):
        b, hp = divmod(core, 4)
        h0 = hp * HPC
        sl = slice(h0 * DK, (h0 + HPC) * DK)

        def wslice(W, scale=None):
            ws = W[sl]  # [128, 512] rows = head outputs
            if scale is not None:
                ws = ws * scale
            # [p, c, h, j] = ws[h*64+j, c*128+p]
            return np.ascontiguousarray(
                ws.reshape(HPC, DK, NCH, 128).transpose(3, 2, 0, 1)
            ).astype(np.float16)

        wo_arr = np.ascontiguousarray(
            W_o[:, sl].T.reshape(HPC, DK, NCH, 128).transpose(1, 0, 2, 3)
        ).astype(np.float16)
        auxa = np.empty((2, 1, HPC, S), np.float16)
        auxa[0] = np.float16(1.0)
        auxa[1, 0, :, :] = pen[b][None, :]
        in_maps.append({
            "qT": xt[("qT", b)],
            "kT": xt[("kT", b)],
            "vT": xt[("vT", b)],
            "wq": wslice(W_q, SCALE),
            "wk": wslice(W_k),
            "wv": wslice(W_v),
            "wo": wo_arr,
            "aux": auxa,
        })
    return in_maps


def kernel(query, key, value, mask, W_q, W_k, W_v, W_o, b_o, _trace=False,
           _trace_kwargs=None):
    query = np.asarray(query, np.float32)
    key = np.asarray(key, np.float32)
    value = np.asarray(value, np.float32)
    mask = np.asarray(mask)
    W_q = np.asarray(W_q, np.float32)
    W_k = np.asarray(W_k, np.float32)
    W_v = np.asarray(W_v, np.float32)
    W_o = np.asarray(W_o, np.float32)
    b_o = np.asarray(b_o, np.float32)

    nc = _get_nc()
    in_maps = _prep_inputs(query, key, value, mask, W_q, W_k, W_v, W_o)
    kw = dict(_trace_kwargs or {})
    res = run_bass_kernel_spmd(nc, in_maps, core_ids=list(range(8)),
                               trace=_trace, **kw)
    kernel.last_result = res

    attn = np.empty((B, H, S, S), np.float32)
    out = np.zeros((B, S, D), np.float32)
    for core in range(8):
        b, hp = divmod(core, 4)
        r = res.results[core]
        for h in range(HPC):
            attn[b, hp * HPC + h] = r["attn_t"][h].reshape(S, S).T
        out[b] += r["out_t"].reshape(D, S).T
    out += b_o
    return out, attn
